# revision 1
# baseline (speedup 1.0000x reference)
"""BlackwellLinear Trainium2 kernel: 2:4 sparsity + int8 fake-quant + x @ w.T + bias.

Full inputs in, full output out. Data-parallel over tokens across 8 NeuronCores;
weight/bias replicated. All module math (sparsify, quantize, matmul, bias) runs
on device; the host only re-encodes layouts: x is transposed, split into exact
fp16 hi/lo planes, and the in_features axis of both x.T and w.T is permuted
phase-major (p <-> 4*(p%256) + p//256). The permutation makes each group-of-4
(the 2:4 sparsity unit) span four k-tiles at the SAME partition/column
coordinates, so the whole sparsify+quantize pipeline is contiguous full-width
elementwise ops and the quantized weight is produced directly in [in_f, out_f]
(lhsT) layout -- no on-device transposes. A contraction-axis permutation
applied to both operands leaves the matmul result unchanged.

Numerics: the reference computes q = round(clip(w_sp / scale)) with
scale = absmax/127 in fp32. There is no float divide on the vector engine, so
the kernel reproduces fl-division bit-exactly (up to ~2^-30 probability edge
cases) with a reciprocal-multiply followed by an exact-residual correction:
  k  = rne(w * inv)                     (magic-constant RNE round)
  d  = (w - k*s_hi) - k*s_lo            (exact: k is a small integer, s split)
  q  = rne(k + d*inv)
clip is a no-op because |w_sp| <= absmax ==> |w_sp/scale| <= 127.00002 < 127.5.
The dequant scale is folded into the PSUM eviction (y = s*(x@q.T) + bias).

Matmul precision: q is an integer <= 127 so it is fp16-exact. x is split as
x = x_hi + x_lo with both halves fp16 (x_hi = fp16(x), x_lo = fp16(x - x_hi);
the residual subtract is exact by Sterbenz, total representation error
~2^-23 |x|). Products x_part * q fit in 18 bits -> exact, accumulated in fp32
PSUM. Two fp16 passes run at 1 cycle/row on the PE -- 2x native fp32 matmul
speed at fp32-envelope accuracy.
"""

import numpy as np

N_CORES = 8
P = 128
IN_F = 1024
OUT_F = 1024
TOKENS = 32768
TOK_PER_CORE = TOKENS // N_CORES  # 4096
K_TILES = IN_F // P  # 8
M_TILES = OUT_F // P  # 8
TB_TOK = 1024  # token block per x strip
N_TB = TOK_PER_CORE // TB_TOK  # 4
MM_N = 512  # matmul moving free dim (one PSUM bank of fp32)
TJ = TB_TOK // MM_N  # matmuls per (mi, ki, part) stationary load

MAGIC = 12582912.0  # 1.5 * 2**23: (v + MAGIC) - MAGIC == RNE round for |v| <= 2**22
SPLIT = 4097.0  # 2**12 + 1: Veltkamp split constant for fp32

# phase-major permutation of the in_features axis: position p holds original
# feature 4*(p%256) + p//256, so k-tile kt covers phase kt//2 of group range
# (kt%2)*128..+128 and the four phases of a group share partition/column coords
_PERM = (4 * (np.arange(IN_F) % 256) + np.arange(IN_F) // 256).astype(np.int64)

_CACHE = {}


def _build(qmax: float):
    from contextlib import ExitStack

    import concourse.tile as tile
    import concourse.mybir as mybir
    from concourse import bacc, bass_isa

    f32 = mybir.dt.float32
    f16 = mybir.dt.float16
    Alu = mybir.AluOpType
    Act = mybir.ActivationFunctionType

    inv_qmax = float(np.float32(1.0) / np.float32(qmax))
    qmaxf = float(np.float32(qmax))

    nc = bacc.Bacc("TRN2", target_bir_lowering=False, debug=False)
    xth = nc.dram_tensor("xth", [IN_F, TOK_PER_CORE], f16, kind="ExternalInput").ap()
    xtl = nc.dram_tensor("xtl", [IN_F, TOK_PER_CORE], f16, kind="ExternalInput").ap()
    # wp: w.T with permuted in_f rows = [in_f_perm, out_f], fp32
    wp = nc.dram_tensor("wp", [IN_F, OUT_F], f32, kind="ExternalInput").ap()
    bias = nc.dram_tensor("bias", [OUT_F], f32, kind="ExternalInput").ap()
    yt = nc.dram_tensor("yt", [OUT_F, TOK_PER_CORE], f32, kind="ExternalOutput").ap()

    with tile.TileContext(nc) as tc, ExitStack() as ctx:
        const = ctx.enter_context(tc.tile_pool(name="const", bufs=1))
        wnat_p = ctx.enter_context(tc.tile_pool(name="wnat", bufs=8))
        abs_p = ctx.enter_context(tc.tile_pool(name="absp", bufs=8))
        thr_p = ctx.enter_context(tc.tile_pool(name="thr", bufs=2))
        thrtmp_p = ctx.enter_context(tc.tile_pool(name="thrtmp", bufs=1))
        scratch = ctx.enter_context(tc.tile_pool(name="scratch", bufs=2))
        qtmp_p = ctx.enter_context(tc.tile_pool(name="qtmp", bufs=2))
        qtmp1_p = ctx.enter_context(tc.tile_pool(name="qtmp1", bufs=1))
        wqt_p = ctx.enter_context(tc.tile_pool(name="wqt", bufs=8))
        sc_p = ctx.enter_context(tc.tile_pool(name="sc", bufs=1))
        x_p = ctx.enter_context(tc.tile_pool(name="x", bufs=9))
        y_p = ctx.enter_context(tc.tile_pool(name="y", bufs=4))
        psum_mm = ctx.enter_context(tc.tile_pool(name="psmm", bufs=8, space="PSUM"))

        # ---- weight load (split across both HWDGE queues for full BW) ----
        wk = [None] * K_TILES
        ak = [None] * K_TILES
        cm = sc_p.tile([P, 8], f32, tag="cm")
        for i, kt in enumerate((0, 1, 2, 3, 4, 5, 6, 7)):
            wt = wnat_p.tile([P, OUT_F], f32, tag="wnat", name=f"wnat{kt}")
            (nc.sync if kt % 2 == 0 else nc.scalar).dma_start(
                wt[:], wp[kt * P : (kt + 1) * P, :]
            )
            wk[kt] = wt
            a = abs_p.tile([P, OUT_F], f32, tag="abs", name=f"abs{kt}")
            nc.scalar.activation(a[:], wt[:], Act.Abs)
            ak[kt] = a
            nc.vector.tensor_reduce(
                out=cm[:, kt : kt + 1],
                in_=a[:],
                axis=mybir.AxisListType.X,
                op=Alu.max,
            )

        # ---- global absmax broadcast to all partitions ----
        amc = sc_p.tile([P, 1], f32, tag="amc")
        nc.vector.reduce_max(amc[:], cm[:], axis=mybir.AxisListType.X)
        am = sc_p.tile([P, 1], f32, tag="am")
        nc.gpsimd.partition_all_reduce(
            am[:], amc[:], channels=P, reduce_op=bass_isa.ReduceOp.max
        )

        # ---- s = fl(absmax/qmax) bit-exact; split s; inv ~= 1/s ----
        _scn = [0]

        def sc_tile():
            _scn[0] += 1
            return sc_p.tile([P, 1], f32, tag=f"sct{_scn[0]}", name=f"sct{_scn[0]}")

        def vts(out, in0, s1, op0, s2=None, op1=None):
            kw = {"op1": op1} if op1 is not None else {}
            nc.vector.tensor_scalar(
                out=out, in0=in0, scalar1=s1, scalar2=s2, op0=op0, **kw
            )

        def vtt(out, in0, in1, op):
            nc.vector.tensor_tensor(out=out, in0=in0, in1=in1, op=op)

        sq0, sc_, stq, shi, slo = (sc_tile() for _ in range(5))
        su, sv, su2, sr, src = (sc_tile() for _ in range(5))
        s_t = sc_p.tile([P, 1], f32, tag="s")
        vts(sq0[:], am[:], inv_qmax, Alu.mult)
        vts(sc_[:], sq0[:], SPLIT, Alu.mult)
        vtt(stq[:], sc_[:], sq0[:], Alu.subtract)
        vtt(shi[:], sc_[:], stq[:], Alu.subtract)
        vtt(slo[:], sq0[:], shi[:], Alu.subtract)
        vts(su[:], shi[:], qmaxf, Alu.mult)
        vtt(sv[:], am[:], su[:], Alu.subtract)
        vts(su2[:], slo[:], qmaxf, Alu.mult)
        vtt(sr[:], sv[:], su2[:], Alu.subtract)
        vts(src[:], sr[:], inv_qmax, Alu.mult)
        vtt(s_t[:], sq0[:], src[:], Alu.add)

        s_hi = sc_p.tile([P, 1], f32, tag="shi")
        s_lo = sc_p.tile([P, 1], f32, tag="slo")
        scs, scts = sc_tile(), sc_tile()
        vts(scs[:], s_t[:], SPLIT, Alu.mult)
        vtt(scts[:], scs[:], s_t[:], Alu.subtract)
        vtt(s_hi[:], scs[:], scts[:], Alu.subtract)
        vtt(s_lo[:], s_t[:], s_hi[:], Alu.subtract)

        inv_t = sc_p.tile([P, 1], f32, tag="inv")
        r0 = sc_tile()
        nc.vector.reciprocal(r0[:], s_t[:])
        for _ in range(2):
            p1, e1, r1 = sc_tile(), sc_tile(), sc_tile()
            vtt(p1[:], s_t[:], r0[:], Alu.mult)
            vts(e1[:], p1[:], 2.0, Alu.subtract)  # p1 - 2 = -(2 - p1)
            vtt(r1[:], r0[:], e1[:], Alu.mult)
            vts(r0[:], r1[:], -1.0, Alu.mult)  # r0 * (2 - p1)
        nc.vector.tensor_copy(inv_t[:], r0[:])
        ninv_t = sc_p.tile([P, 1], f32, tag="ninv")
        vts(ninv_t[:], inv_t[:], -1.0, Alu.mult)
        magic_t = sc_p.tile([P, 1], f32, tag="magic")
        nc.gpsimd.memset(magic_t[:], MAGIC)
        nmagic_t = sc_p.tile([P, 1], f32, tag="nmagic")
        nc.gpsimd.memset(nmagic_t[:], -MAGIC)
        one_t = sc_p.tile([P, 1], f32, tag="one")
        nc.gpsimd.memset(one_t[:], 1.0)

        # ---- bias slices ----
        bias_t = []
        for mi in range(M_TILES):
            bt = const.tile([P, 1], f32, tag=f"bias{mi}")
            nc.sync.dma_start(bt[:, 0:1], bias[mi * P : (mi + 1) * P].unsqueeze(1))
            bias_t.append(bt)

        # ---- 2:4 threshold per group-range (contiguous, phases = k-tiles) ----
        # thr_r = 2nd largest |w| of each group = max(min of pair maxes,
        # max of pair mins) over the 4 phase tiles of range r
        def build_thr(r):
            a0, a1, a2, a3 = (ak[2 * j + r] for j in range(4))
            tA = thrtmp_p.tile([P, OUT_F], f32, tag="tA", name=f"tA_{r}")
            tB = thrtmp_p.tile([P, OUT_F], f32, tag="tB", name=f"tB_{r}")
            tC = thrtmp_p.tile([P, OUT_F], f32, tag="tC", name=f"tC_{r}")
            tr = thr_p.tile([P, OUT_F], f32, tag="thr", name=f"thr_{r}")
            vtt(tA[:], a0[:], a1[:], Alu.max)
            vtt(tB[:], a2[:], a3[:], Alu.max)
            vtt(tA[:], tA[:], tB[:], Alu.min)  # t1 = min of pair maxes
            vtt(tB[:], a0[:], a1[:], Alu.min)
            vtt(tC[:], a2[:], a3[:], Alu.min)
            vtt(tB[:], tB[:], tC[:], Alu.max)  # t2 = max of pair mins
            vtt(tr[:], tA[:], tB[:], Alu.max)
            return tr

        # ---- per k-tile: quantize w directly (rounding commutes with the
        # sparsity mask elementwise), mask in parallel, combine at the end.
        # q16 k-tiles land directly in lhsT [in_f, out_f] layout.
        # emission order drives Tile's scheduling priority: put k-tile 0's
        # whole chain (thr range 0 -> quant -> mask) ahead of everything else
        # so the PE's first stationary tile lands as early as possible
        wqt_by_kt = {}
        thr_cache = {}
        kt_order = list(range(K_TILES))
        for kt in kt_order:
            r = kt % 2
            if r not in thr_cache:
                thr_cache[r] = build_thr(r)
            wt, a, tr = wk[kt], ak[kt], thr_cache[r]
            m = scratch.tile([P, OUT_F], f32, tag="mask")
            vtt(m[:], a[:], tr[:], Alu.is_ge)

            q0 = qtmp_p.tile([P, OUT_F], f32, tag="q0")
            k = qtmp_p.tile([P, OUT_F], f32, tag="k")
            n1 = qtmp1_p.tile([P, OUT_F], f32, tag="n1")
            n2 = qtmp1_p.tile([P, OUT_F], f32, tag="n2")
            # k = rne(w * inv) via the magic constant (ACT: in*scale + bias)
            nc.scalar.activation(
                q0[:], wt[:], Act.Identity, bias=magic_t[:], scale=inv_t[:]
            )
            nc.scalar.activation(
                k[:], q0[:], Act.Identity, bias=nmagic_t[:], scale=one_t[:]
            )
            # exact residual: n2 = k*s - w (k integer, s split => exact)
            nc.vector.scalar_tensor_tensor(
                out=n1[:], in0=k[:], scalar=s_hi[:], in1=wt[:],
                op0=Alu.mult, op1=Alu.subtract,
            )
            nc.vector.scalar_tensor_tensor(
                out=n2[:], in0=k[:], scalar=s_lo[:], in1=n1[:],
                op0=Alu.mult, op1=Alu.add,
            )
            # v = k + (w - k*s)*inv = k + n2*(-inv)
            nc.vector.scalar_tensor_tensor(
                out=q0[:], in0=n2[:], scalar=ninv_t[:], in1=k[:],
                op0=Alu.mult, op1=Alu.add,
            )
            vts(q0[:], q0[:], MAGIC, Alu.add, MAGIC, Alu.subtract)  # q = rne(v)
            vtt(q0[:], q0[:], m[:], Alu.mult)  # apply 2:4 mask
            q16 = wqt_p.tile([P, OUT_F], f16, tag="q16", name=f"q16_{kt}")
            nc.scalar.copy(q16[:], q0[:])
            wqt_by_kt[kt] = q16
        wqt = [wqt_by_kt[kt] for kt in range(K_TILES)]

        # ---- main matmul: yt[m, t] = sum_k wqt[k,m].T @ (xh[k,t] + xl[k,t]) ----
        # tb0 x loads share the sync queue (after w); later tbs go on the ACT
        # queue and self-throttle via pool backpressure; out stores on ACT queue
        for tb in range(N_TB):
            dma_eng = nc.sync if tb == 0 else nc.scalar
            xh, xl = [], []
            for ki in range(K_TILES):
                sl_p = slice(ki * P, (ki + 1) * P)
                sl_t = slice(tb * TB_TOK, (tb + 1) * TB_TOK)
                xht = x_p.tile([P, TB_TOK], f16, tag="xh", name=f"xh{tb}_{ki}")
                dma_eng.dma_start(xht[:], xth[sl_p, sl_t])
                xlt = x_p.tile([P, TB_TOK], f16, tag="xl", name=f"xl{tb}_{ki}")
                dma_eng.dma_start(xlt[:], xtl[sl_p, sl_t])
                xh.append(xht)
                xl.append(xlt)
            def evict(mi, ps_tj):
                for tj in range(TJ):
                    ysb = y_p.tile([P, MM_N], f32, tag="ysb", name=f"y{tb}_{mi}_{tj}")
                    nc.scalar.activation(
                        ysb[:],
                        ps_tj[tj][:],
                        Act.Identity,
                        bias=bias_t[mi][:],
                        scale=s_t[:],
                    )
                    tcol = tb * TB_TOK + tj * MM_N
                    nc.scalar.dma_start(
                        yt[mi * P : (mi + 1) * P, tcol : tcol + MM_N], ysb[:]
                    )

            if tb == 0:
                # k-outer sweep: PE starts as soon as the first quantized
                # k-tile lands, consuming k-tiles at the prep pipeline's pace
                for mh in range(2):
                    ps = {
                        (ml, tj): psum_mm.tile(
                            [P, MM_N], f32, tag="ps", name=f"ps0_{mh}_{ml}_{tj}"
                        )
                        for ml in range(4)
                        for tj in range(TJ)
                    }
                    for ki in range(K_TILES):
                        for ml in range(4):
                            mi = mh * 4 + ml
                            lhsT = wqt[ki][:, mi * P : (mi + 1) * P]
                            for part, xp in ((0, xh), (1, xl)):
                                for tj in range(TJ):
                                    nc.tensor.matmul(
                                        ps[ml, tj][:],
                                        lhsT,
                                        xp[ki][:, tj * MM_N : (tj + 1) * MM_N],
                                        start=(ki == 0 and part == 0),
                                        stop=(ki == K_TILES - 1 and part == 1),
                                    )
                    for ml in range(4):
                        evict(mh * 4 + ml, [ps[ml, tj] for tj in range(TJ)])
            else:
                for mi in range(M_TILES):
                    ps = [
                        psum_mm.tile(
                            [P, MM_N], f32, tag="ps", name=f"ps{tb}_{mi}_{tj}"
                        )
                        for tj in range(TJ)
                    ]
                    for ki in range(K_TILES):
                        lhsT = wqt[ki][:, mi * P : (mi + 1) * P]
                        for part, xp in ((0, xh), (1, xl)):
                            for tj in range(TJ):
                                nc.tensor.matmul(
                                    ps[tj][:],
                                    lhsT,
                                    xp[ki][:, tj * MM_N : (tj + 1) * MM_N],
                                    start=(ki == 0 and part == 0),
                                    stop=(ki == K_TILES - 1 and part == 1),
                                )
                    evict(mi, ps)

    nc.compile()
    return nc


def _get(qmax: float):
    key = qmax
    if key not in _CACHE:
        _CACHE[key] = _build(qmax)
    return _CACHE[key]


def host_prep(x, weight):
    """Host-side input re-encoding: transpose, phase-major permute the in_f
    axis, exact fp16 hi/lo split of x. Pure layout/encoding; no module math."""
    xt = np.ascontiguousarray(x.T)[_PERM]  # [IN_F perm, TOKENS]
    xth = xt.astype(np.float16)
    xtl = (xt - xth.astype(np.float32)).astype(np.float16)
    wp = np.ascontiguousarray(weight.T[_PERM])  # [IN_F perm, OUT_F]
    return xth, xtl, wp


LAST_EXEC_NS = None


def kernel(x, weight, bias, precision, _trace_dir=None):
    global LAST_EXEC_NS
    from concourse.bass_utils import run_bass_kernel_spmd

    x = np.asarray(x, dtype=np.float32)
    weight = np.asarray(weight, dtype=np.float32)
    bias = np.asarray(bias, dtype=np.float32)
    prec = int(np.asarray(precision))
    qmax = float(2 ** (prec - 1) - 1)

    nc = _get(qmax)

    xth, xtl, wp = host_prep(x, weight)
    in_maps = [
        {
            "xth": np.ascontiguousarray(
                xth[:, c * TOK_PER_CORE : (c + 1) * TOK_PER_CORE]
            ),
            "xtl": np.ascontiguousarray(
                xtl[:, c * TOK_PER_CORE : (c + 1) * TOK_PER_CORE]
            ),
            "wp": wp,
            "bias": bias,
        }
        for c in range(N_CORES)
    ]
    kw = {}
    if _trace_dir is not None:
        kw = {"trace": True, "tmpdir": _trace_dir}
    res = run_bass_kernel_spmd(nc, in_maps, list(range(N_CORES)), **kw)
    LAST_EXEC_NS = res.exec_time_ns
    yt = np.concatenate([res.results[c]["yt"] for c in range(N_CORES)], axis=1)
    return np.ascontiguousarray(yt.T)



# revision 2
# speedup vs baseline: 1.7517x; 1.7517x over previous
"""BlackwellLinear Trainium2 kernel: 2:4 sparsity + int8 fake-quant + x @ w.T + bias.

Full inputs in, full output out. Data-parallel over tokens across 8 NeuronCores;
weight/bias replicated. All module math (sparsify, quantize, matmul, bias) runs
on device; the host only re-encodes layouts: x is transposed to fp16, and the
in_features axis of both x.T and w.T is permuted phase-major
(p <-> 4*(p%256) + p//256), so each group-of-4 (the 2:4 sparsity unit) spans
four k-tiles at the SAME partition/column coordinates. The sparsify+quantize
pipeline is then contiguous full-width elementwise ops and the quantized weight
is produced directly in [in_f, out_f] (lhsT) layout -- no on-device transposes.
A contraction-axis permutation applied to both operands leaves the matmul
result unchanged.

Numerics (target: harness rel-err gate 2e-2; this kernel lands ~6e-4):
  s   = absmax * (1/qmax)                  (fp32)
  inv ~= 1/s                               (reciprocal + 1 Newton, ~6e-8 rel)
  q   = rne(w * inv) via the magic-constant trick; clip is a no-op because
        |w| <= absmax ==> |w * inv| <= qmax*(1+eps) < qmax + 0.5.
  y   = s * (x16 @ (q * mask).T) + bias    (scale folded into PSUM eviction)
x is sent as a single fp16 plane (~2^-11 relative rounding -> ~3e-4 on y);
q <= 127 is fp16-exact; accumulation is fp32 in PSUM. y is evicted as fp16
(~2.4e-4) and widened to fp32 on the host. One fp16 matmul pass runs at
1 col/cycle on the PE -- half the work of the previous hi/lo two-pass design.

k-tile order is evens-first (0,2,4,6,1,3,5,7): the 2:4 threshold for group
range 0 only needs the abs of k-tiles {0,2,4,6}, so the PE's first
accumulation sweep can start after one threshold build instead of two.
"""

import numpy as np

N_CORES = 8
P = 128
IN_F = 1024
OUT_F = 1024
TOKENS = 32768
TOK_PER_CORE = TOKENS // N_CORES  # 4096
K_TILES = IN_F // P  # 8
M_TILES = OUT_F // P  # 8
TB_TOK = 1024  # token block per x strip
N_TB = TOK_PER_CORE // TB_TOK  # 4
MM_N = 512  # matmul moving free dim (one PSUM bank of fp32)
TJ = TB_TOK // MM_N  # matmuls per (mi, ki) stationary load

MAGIC = 12582912.0  # 1.5 * 2**23: (v + MAGIC) - MAGIC == RNE round for |v| <= 2**22

# evens first: range-0 threshold (k-tiles 0,2,4,6) unblocks 4 quant chains
KT_ORDER = (0, 2, 4, 6, 1, 3, 5, 7)

# phase-major permutation of the in_features axis: position p holds original
# feature 4*(p%256) + p//256, so k-tile kt covers phase kt//2 of group range
# (kt%2)*128..+128 and the four phases of a group share partition/column coords
_PERM = (4 * (np.arange(IN_F) % 256) + np.arange(IN_F) // 256).astype(np.int64)

_CACHE = {}


def _build(qmax: float):
    from contextlib import ExitStack

    import concourse.tile as tile
    import concourse.mybir as mybir
    from concourse import bacc, bass_isa

    f32 = mybir.dt.float32
    f16 = mybir.dt.float16
    Alu = mybir.AluOpType
    Act = mybir.ActivationFunctionType

    inv_qmax = float(np.float32(1.0) / np.float32(qmax))

    nc = bacc.Bacc("TRN2", target_bir_lowering=False, debug=False)
    xth = nc.dram_tensor("xth", [IN_F, TOK_PER_CORE], f16, kind="ExternalInput").ap()
    # wp: w.T with permuted in_f rows = [in_f_perm, out_f], fp32
    wp = nc.dram_tensor("wp", [IN_F, OUT_F], f32, kind="ExternalInput").ap()
    bias = nc.dram_tensor("bias", [OUT_F], f32, kind="ExternalInput").ap()
    yt = nc.dram_tensor("yt", [OUT_F, TOK_PER_CORE], f16, kind="ExternalOutput").ap()

    with tile.TileContext(nc) as tc, ExitStack() as ctx:
        const = ctx.enter_context(tc.tile_pool(name="const", bufs=1))
        wnat_p = ctx.enter_context(tc.tile_pool(name="wnat", bufs=8))
        abs_p = ctx.enter_context(tc.tile_pool(name="absp", bufs=8))
        thr_p = ctx.enter_context(tc.tile_pool(name="thr", bufs=2))
        thrtmp_p = ctx.enter_context(tc.tile_pool(name="thrtmp", bufs=1))
        scratch = ctx.enter_context(tc.tile_pool(name="scratch", bufs=2))
        qtmp_p = ctx.enter_context(tc.tile_pool(name="qtmp", bufs=2))
        wqt_p = ctx.enter_context(tc.tile_pool(name="wqt", bufs=8))
        sc_p = ctx.enter_context(tc.tile_pool(name="sc", bufs=1))
        x_p = ctx.enter_context(tc.tile_pool(name="x", bufs=9))
        y_p = ctx.enter_context(tc.tile_pool(name="y", bufs=4))
        psum_mm = ctx.enter_context(tc.tile_pool(name="psmm", bufs=8, space="PSUM"))

        def vts(out, in0, s1, op0, s2=None, op1=None):
            kw = {"op1": op1} if op1 is not None else {}
            nc.vector.tensor_scalar(
                out=out, in0=in0, scalar1=s1, scalar2=s2, op0=op0, **kw
            )

        def vtt(out, in0, in1, op):
            nc.vector.tensor_tensor(out=out, in0=in0, in1=in1, op=op)

        # ---- weight load (split across both HWDGE queues for full BW) ----
        wk = [None] * K_TILES
        ak = [None] * K_TILES
        cm = sc_p.tile([P, 8], f32, tag="cm")
        for i, kt in enumerate(KT_ORDER):
            wt = wnat_p.tile([P, OUT_F], f32, tag="wnat", name=f"wnat{kt}")
            (nc.sync if i % 2 == 0 else nc.scalar).dma_start(
                wt[:], wp[kt * P : (kt + 1) * P, :]
            )
            wk[kt] = wt
            a = abs_p.tile([P, OUT_F], f32, tag="abs", name=f"abs{kt}")
            nc.scalar.activation(a[:], wt[:], Act.Abs)
            ak[kt] = a
            nc.vector.tensor_reduce(
                out=cm[:, kt : kt + 1],
                in_=a[:],
                axis=mybir.AxisListType.X,
                op=Alu.max,
            )

        # ---- global absmax broadcast to all partitions ----
        amc = sc_p.tile([P, 1], f32, tag="amc")
        nc.vector.reduce_max(amc[:], cm[:], axis=mybir.AxisListType.X)
        am = sc_p.tile([P, 1], f32, tag="am")
        nc.gpsimd.partition_all_reduce(
            am[:], amc[:], channels=P, reduce_op=bass_isa.ReduceOp.max
        )

        # ---- s = absmax/qmax; inv ~= 1/s (reciprocal + 1 Newton step) ----
        s_t = sc_p.tile([P, 1], f32, tag="s")
        vts(s_t[:], am[:], inv_qmax, Alu.mult)
        r0 = sc_p.tile([P, 1], f32, tag="r0")
        nc.vector.reciprocal(r0[:], s_t[:])
        p1 = sc_p.tile([P, 1], f32, tag="p1")
        e1 = sc_p.tile([P, 1], f32, tag="e1")
        r1 = sc_p.tile([P, 1], f32, tag="r1")
        inv_t = sc_p.tile([P, 1], f32, tag="inv")
        vtt(p1[:], s_t[:], r0[:], Alu.mult)
        vts(e1[:], p1[:], 2.0, Alu.subtract)  # p1 - 2 = -(2 - p1)
        vtt(r1[:], r0[:], e1[:], Alu.mult)  # -r0*(2 - p1)
        vts(inv_t[:], r1[:], -1.0, Alu.mult)
        magic_t = sc_p.tile([P, 1], f32, tag="magic")
        nc.gpsimd.memset(magic_t[:], MAGIC)

        # ---- bias slices ----
        bias_t = []
        for mi in range(M_TILES):
            bt = const.tile([P, 1], f32, tag=f"bias{mi}")
            nc.sync.dma_start(bt[:, 0:1], bias[mi * P : (mi + 1) * P].unsqueeze(1))
            bias_t.append(bt)

        # ---- 2:4 threshold per group-range (contiguous, phases = k-tiles) ----
        # thr_r = 2nd largest |w| of each group = max(min of pair maxes,
        # max of pair mins) over the 4 phase tiles of range r
        def build_thr(r):
            a0, a1, a2, a3 = (ak[2 * j + r] for j in range(4))
            tA = thrtmp_p.tile([P, OUT_F], f32, tag="tA", name=f"tA_{r}")
            tB = thrtmp_p.tile([P, OUT_F], f32, tag="tB", name=f"tB_{r}")
            tC = thrtmp_p.tile([P, OUT_F], f32, tag="tC", name=f"tC_{r}")
            tr = thr_p.tile([P, OUT_F], f32, tag="thr", name=f"thr_{r}")
            vtt(tA[:], a0[:], a1[:], Alu.max)
            vtt(tB[:], a2[:], a3[:], Alu.max)
            vtt(tA[:], tA[:], tB[:], Alu.min)  # t1 = min of pair maxes
            vtt(tB[:], a0[:], a1[:], Alu.min)
            vtt(tC[:], a2[:], a3[:], Alu.min)
            vtt(tB[:], tB[:], tC[:], Alu.max)  # t2 = max of pair mins
            vtt(tr[:], tA[:], tB[:], Alu.max)
            return tr

        # ---- per k-tile: quantize w (rounding commutes with the sparsity
        # mask elementwise), mask via is_ge, combine + downcast in one DVE op.
        # q16 k-tiles land directly in lhsT [in_f, out_f] layout.
        wqt_by_kt = {}
        thr_cache = {}
        for kt in KT_ORDER:
            r = kt % 2
            if r not in thr_cache:
                thr_cache[r] = build_thr(r)
            wt, a, tr = wk[kt], ak[kt], thr_cache[r]
            m = scratch.tile([P, OUT_F], f32, tag="mask")
            vtt(m[:], a[:], tr[:], Alu.is_ge)
            q0 = qtmp_p.tile([P, OUT_F], f32, tag="q0")
            # q0 = rne(w * inv) + MAGIC (ACT: in*scale + bias, fp32)
            nc.scalar.activation(
                q0[:], wt[:], Act.Identity, bias=magic_t[:], scale=inv_t[:]
            )
            # q16 = (q0 - MAGIC) * m, downcast to fp16 on the write port
            q16 = wqt_p.tile([P, OUT_F], f16, tag="q16", name=f"q16_{kt}")
            nc.vector.scalar_tensor_tensor(
                out=q16[:], in0=q0[:], scalar=-MAGIC, in1=m[:],
                op0=Alu.add, op1=Alu.mult,
            )
            wqt_by_kt[kt] = q16
        wqt = [wqt_by_kt[kt] for kt in range(K_TILES)]

        # ---- main matmul: yt[m, t] = sum_k wqt[k,m].T @ x16[k,t] ----
        # tb0 x loads share the sync queue (after w); later tbs go on the ACT
        # queue and self-throttle via pool backpressure; out stores on sync
        for tb in range(N_TB):
            dma_eng = nc.sync if tb == 0 else nc.scalar
            xh = {}
            for ki in KT_ORDER:
                sl_p = slice(ki * P, (ki + 1) * P)
                sl_t = slice(tb * TB_TOK, (tb + 1) * TB_TOK)
                xht = x_p.tile([P, TB_TOK], f16, tag="xh", name=f"xh{tb}_{ki}")
                dma_eng.dma_start(xht[:], xth[sl_p, sl_t])
                xh[ki] = xht

            def evict(mi, ps_tj, eng):
                for tj in range(TJ):
                    ysb = y_p.tile([P, MM_N], f16, tag="ysb", name=f"y{tb}_{mi}_{tj}")
                    if eng == 0:
                        nc.scalar.activation(
                            ysb[:],
                            ps_tj[tj][:],
                            Act.Identity,
                            bias=bias_t[mi][:],
                            scale=s_t[:],
                        )
                    else:
                        nc.vector.tensor_scalar(
                            out=ysb[:],
                            in0=ps_tj[tj][:],
                            scalar1=s_t[:],
                            scalar2=bias_t[mi][:],
                            op0=Alu.mult,
                            op1=Alu.add,
                        )
                    tcol = tb * TB_TOK + tj * MM_N
                    nc.sync.dma_start(
                        yt[mi * P : (mi + 1) * P, tcol : tcol + MM_N], ysb[:]
                    )

            if tb == 0:
                # k-outer sweep: PE starts as soon as the first quantized
                # k-tile lands, consuming k-tiles at the prep pipeline's pace
                for mh in range(2):
                    ps = {
                        (ml, tj): psum_mm.tile(
                            [P, MM_N], f32, tag="ps", name=f"ps0_{mh}_{ml}_{tj}"
                        )
                        for ml in range(4)
                        for tj in range(TJ)
                    }
                    for i, ki in enumerate(KT_ORDER):
                        for ml in range(4):
                            mi = mh * 4 + ml
                            lhsT = wqt[ki][:, mi * P : (mi + 1) * P]
                            for tj in range(TJ):
                                nc.tensor.matmul(
                                    ps[ml, tj][:],
                                    lhsT,
                                    xh[ki][:, tj * MM_N : (tj + 1) * MM_N],
                                    start=(i == 0),
                                    stop=(i == K_TILES - 1),
                                )
                    for ml in range(4):
                        mi = mh * 4 + ml
                        evict(mi, [ps[ml, tj] for tj in range(TJ)], eng=mi % 2)
            else:
                for mi in range(M_TILES):
                    ps = [
                        psum_mm.tile(
                            [P, MM_N], f32, tag="ps", name=f"ps{tb}_{mi}_{tj}"
                        )
                        for tj in range(TJ)
                    ]
                    for i, ki in enumerate(KT_ORDER):
                        lhsT = wqt[ki][:, mi * P : (mi + 1) * P]
                        for tj in range(TJ):
                            nc.tensor.matmul(
                                ps[tj][:],
                                lhsT,
                                xh[ki][:, tj * MM_N : (tj + 1) * MM_N],
                                start=(i == 0),
                                stop=(i == K_TILES - 1),
                            )
                    evict(mi, ps, eng=(mi + tb) % 2)

    nc.compile()
    return nc


def _get(qmax: float):
    key = qmax
    if key not in _CACHE:
        _CACHE[key] = _build(qmax)
    return _CACHE[key]


def host_prep(x, weight):
    """Host-side input re-encoding: transpose, phase-major permute the in_f
    axis, fp16 downcast of x. Pure layout/encoding; no module math."""
    xt = np.ascontiguousarray(x.T)[_PERM]  # [IN_F perm, TOKENS]
    xth = xt.astype(np.float16)
    wp = np.ascontiguousarray(weight.T[_PERM])  # [IN_F perm, OUT_F]
    return xth, wp


LAST_EXEC_NS = None


def kernel(x, weight, bias, precision, _trace_dir=None):
    global LAST_EXEC_NS
    from concourse.bass_utils import run_bass_kernel_spmd

    x = np.asarray(x, dtype=np.float32)
    weight = np.asarray(weight, dtype=np.float32)
    bias = np.asarray(bias, dtype=np.float32)
    prec = int(np.asarray(precision))
    qmax = float(2 ** (prec - 1) - 1)

    nc = _get(qmax)

    xth, wp = host_prep(x, weight)
    in_maps = [
        {
            "xth": np.ascontiguousarray(
                xth[:, c * TOK_PER_CORE : (c + 1) * TOK_PER_CORE]
            ),
            "wp": wp,
            "bias": bias,
        }
        for c in range(N_CORES)
    ]
    kw = {}
    if _trace_dir is not None:
        kw = {"trace": True, "tmpdir": _trace_dir}
    res = run_bass_kernel_spmd(nc, in_maps, list(range(N_CORES)), **kw)
    LAST_EXEC_NS = res.exec_time_ns
    yt = np.concatenate([res.results[c]["yt"] for c in range(N_CORES)], axis=1)
    return np.ascontiguousarray(yt.T.astype(np.float32))


# revision 6
# speedup vs baseline: 1.8559x; 1.0595x over previous
"""BlackwellLinear Trainium2 kernel: 2:4 sparsity + int8 fake-quant + x @ w.T + bias.

Full inputs in, full output out. Data-parallel over tokens across 8 NeuronCores;
weight/bias replicated. All module math (sparsify, quantize, matmul, bias) runs
on device; the host only re-encodes layouts: x is transposed to fp16, and the
in_features axis of both x.T and w.T is permuted phase-major
(p <-> 4*(p%256) + p//256), so each group-of-4 (the 2:4 sparsity unit) spans
four k-tiles at the SAME partition/column coordinates. The sparsify+quantize
pipeline is then contiguous full-width elementwise ops and the quantized weight
is produced directly in [in_f, out_f] (lhsT) layout -- no on-device transposes.

Numerics (harness rel-err gate 2e-2; this kernel lands ~1e-3):
  s   = absmax * (1/qmax)                  (fp32)
  inv ~= 1/s = qmax/absmax                 (reciprocal + 1 Newton on absmax)
  q   = rne(w * inv) via the magic-constant trick; clip is a no-op because
        |w| <= absmax ==> |w * inv| <= qmax*(1+eps) < qmax + 0.5.
  y   = s * (x16 @ (q * mask).T) + bias    (scale folded into PSUM eviction)
x is a single fp16 plane (~2^-11 rounding -> ~3e-4 on y); q <= 127 is
fp16-exact; accumulation is fp32 in PSUM; y is evicted fp16 and widened on the
host. The 2:4 threshold compare MUST run in fp32: fp16 |w| creates rounding
ties where the 3rd-largest of a group equals the 2nd, so is_ge keeps 3
elements and injects full-magnitude spurious weights (measured 5e-2 rel err).

Ramp design (the PE can only start after absmax -> inv -> first quantized
k-tile): all 8 w DMAs are issued before any compute so neither HWDGE queue
stalls behind engine work; per-k-tile column maxes pipeline with DMA arrivals;
k-tiles are consumed evens-first so one threshold build unblocks the PE's
first 4-k-tile sweep while range 1's threshold builds behind it.
"""

import numpy as np

N_CORES = 8
P = 128
IN_F = 1024
OUT_F = 1024
TOKENS = 32768
TOK_PER_CORE = TOKENS // N_CORES  # 4096
K_TILES = IN_F // P  # 8
M_TILES = OUT_F // P  # 8
TB_TOK = 2048  # token block per x strip
N_TB = TOK_PER_CORE // TB_TOK  # 2
MM_N = 512  # matmul moving free dim (one PSUM bank of fp32)
TJ = TB_TOK // MM_N  # 4 matmuls per (mi, ki) stationary load

MAGIC = 12582912.0  # 1.5 * 2**23: (v + MAGIC) - MAGIC == RNE round for |v| <= 2**22

# evens first: range-0 threshold (k-tiles 0,2,4,6) unblocks 4 quant chains
KT_ORDER = (0, 2, 4, 6, 1, 3, 5, 7)

# phase-major permutation of the in_features axis: position p holds original
# feature 4*(p%256) + p//256, so k-tile kt covers phase kt//2 of group range
# (kt%2)*128..+128 and the four phases of a group share partition/column coords
_PERM = (4 * (np.arange(IN_F) % 256) + np.arange(IN_F) // 256).astype(np.int64)

_CACHE = {}


def _build(qmax: float):
    from contextlib import ExitStack

    import concourse.tile as tile
    import concourse.mybir as mybir
    from concourse import bacc, bass_isa

    f32 = mybir.dt.float32
    f16 = mybir.dt.float16
    Alu = mybir.AluOpType
    Act = mybir.ActivationFunctionType

    inv_qmax = float(np.float32(1.0) / np.float32(qmax))
    nqmaxf = -float(np.float32(qmax))

    nc = bacc.Bacc("TRN2", target_bir_lowering=False, debug=False)
    xth = nc.dram_tensor("xth", [IN_F, TOK_PER_CORE], f16, kind="ExternalInput").ap()
    # wp: w.T with permuted in_f rows = [in_f_perm, out_f], fp32
    wp = nc.dram_tensor("wp", [IN_F, OUT_F], f32, kind="ExternalInput").ap()
    bias = nc.dram_tensor("bias", [OUT_F], f32, kind="ExternalInput").ap()
    yt = nc.dram_tensor("yt", [OUT_F, TOK_PER_CORE], f16, kind="ExternalOutput").ap()

    with tile.TileContext(nc) as tc, ExitStack() as ctx:
        const = ctx.enter_context(tc.tile_pool(name="const", bufs=1))
        wnat_p = ctx.enter_context(tc.tile_pool(name="wnat", bufs=8))
        abs_p = ctx.enter_context(tc.tile_pool(name="absp", bufs=8))
        tree_p = ctx.enter_context(tc.tile_pool(name="tree", bufs=4))
        thr_p = ctx.enter_context(tc.tile_pool(name="thr", bufs=2))
        t12_p = ctx.enter_context(tc.tile_pool(name="t12", bufs=1))
        mins_p = ctx.enter_context(tc.tile_pool(name="mins", bufs=2))
        mask_p = ctx.enter_context(tc.tile_pool(name="mask", bufs=3))
        qtmp_p = ctx.enter_context(tc.tile_pool(name="qtmp", bufs=2))
        wqt_p = ctx.enter_context(tc.tile_pool(name="wqt", bufs=8))
        sc_p = ctx.enter_context(tc.tile_pool(name="sc", bufs=1))
        x_p = ctx.enter_context(tc.tile_pool(name="x", bufs=11))
        y_p = ctx.enter_context(tc.tile_pool(name="y", bufs=3))
        psum_mm = ctx.enter_context(tc.tile_pool(name="psmm", bufs=8, space="PSUM"))

        def vts(out, in0, s1, op0, s2=None, op1=None):
            kw = {"op1": op1} if op1 is not None else {}
            nc.vector.tensor_scalar(
                out=out, in0=in0, scalar1=s1, scalar2=s2, op0=op0, **kw
            )

        def vtt(out, in0, in1, op):
            nc.vector.tensor_tensor(out=out, in0=in0, in1=in1, op=op)

        # ---- all weight DMAs first, split across both HWDGE queues ----
        wk = [None] * K_TILES
        for i, kt in enumerate(KT_ORDER):
            wt = wnat_p.tile([P, OUT_F], f32, tag="wnat", name=f"wnat{kt}")
            (nc.sync if i % 2 == 0 else nc.scalar).dma_start(
                wt[:], wp[kt * P : (kt + 1) * P, :]
            )
            wk[kt] = wt
        magic_t = sc_p.tile([P, 1], f32, tag="magic")
        nc.gpsimd.memset(magic_t[:], MAGIC)

        # ---- |w| (ACT) + per-tile column max (DVE), pipelined with arrivals
        ak = [None] * K_TILES
        cm = sc_p.tile([P, 8], f32, tag="cm")
        for kt in KT_ORDER:
            a = abs_p.tile([P, OUT_F], f32, tag="abs", name=f"abs{kt}")
            nc.scalar.activation(a[:], wk[kt][:], Act.Abs)
            ak[kt] = a
            nc.vector.tensor_reduce(
                out=cm[:, kt : kt + 1],
                in_=a[:],
                axis=mybir.AxisListType.X,
                op=Alu.max,
            )

        # ---- thr_0 build goes ahead of the global absmax combine: it only
        # needs the even k-tiles and unblocks the PE's first sweep ----
        def build_thr(r):
            tA = tree_p.tile([P, OUT_F], f32, tag="tAB", name=f"tA_{r}")
            vtt(tA[:], ak[r][:], ak[r + 2][:], Alu.max)
            tB = tree_p.tile([P, OUT_F], f32, tag="tAB", name=f"tB_{r}")
            vtt(tB[:], ak[r + 4][:], ak[r + 6][:], Alu.max)
            t1 = t12_p.tile([P, OUT_F], f32, tag="t1", name=f"t1_{r}")
            vtt(t1[:], tA[:], tB[:], Alu.min)
            mA = mins_p.tile([P, OUT_F], f32, tag="mins", name=f"mA_{r}")
            vtt(mA[:], ak[r][:], ak[r + 2][:], Alu.min)
            mB = mins_p.tile([P, OUT_F], f32, tag="mins", name=f"mB_{r}")
            vtt(mB[:], ak[r + 4][:], ak[r + 6][:], Alu.min)
            t2 = t12_p.tile([P, OUT_F], f32, tag="t2", name=f"t2_{r}")
            vtt(t2[:], mA[:], mB[:], Alu.max)
            tr = thr_p.tile([P, OUT_F], f32, tag="thr", name=f"thr_{r}")
            vtt(tr[:], t1[:], t2[:], Alu.max)
            return tr

        thr0 = build_thr(0)

        # ---- global absmax -> inv ~= qmax/absmax (recip + 1 Newton) ----
        amc = sc_p.tile([P, 1], f32, tag="amc")
        nc.vector.reduce_max(amc[:], cm[:], axis=mybir.AxisListType.X)
        am = sc_p.tile([P, 1], f32, tag="am")
        nc.gpsimd.partition_all_reduce(
            am[:], amc[:], channels=P, reduce_op=bass_isa.ReduceOp.max
        )
        r0 = sc_p.tile([P, 1], f32, tag="r0")
        nc.vector.reciprocal(r0[:], am[:])
        p1 = sc_p.tile([P, 1], f32, tag="p1")
        e1 = sc_p.tile([P, 1], f32, tag="e1")
        r1 = sc_p.tile([P, 1], f32, tag="r1")
        inv_t = sc_p.tile([P, 1], f32, tag="inv")
        vtt(p1[:], am[:], r0[:], Alu.mult)
        vts(e1[:], p1[:], 2.0, Alu.subtract)  # p1 - 2 = -(2 - p1)
        vtt(r1[:], r0[:], e1[:], Alu.mult)  # -r0*(2 - p1) ~= -1/absmax
        vts(inv_t[:], r1[:], nqmaxf, Alu.mult)  # qmax/absmax
        s_t = sc_p.tile([P, 1], f32, tag="s")
        vts(s_t[:], am[:], inv_qmax, Alu.mult)

        # ---- per k-tile (evens first): mask (DVE f32 compare -> f16), q0 =
        # rne(w*inv)+MAGIC (ACT), q16 = (q0 - MAGIC)*m downcast fp16 (DVE) ----
        wqt_by_kt = {}
        thr_cache = {0: thr0}
        for kt in KT_ORDER:
            r = kt % 2
            if r not in thr_cache:
                thr_cache[r] = build_thr(r)
            m = mask_p.tile([P, OUT_F], f16, tag="mask", name=f"m_{kt}")
            vtt(m[:], ak[kt][:], thr_cache[r][:], Alu.is_ge)
            q0 = qtmp_p.tile([P, OUT_F], f32, tag="q0", name=f"q0_{kt}")
            nc.scalar.activation(
                q0[:], wk[kt][:], Act.Identity, bias=magic_t[:], scale=inv_t[:]
            )
            q16 = wqt_p.tile([P, OUT_F], f16, tag="q16", name=f"q16_{kt}")
            nc.vector.scalar_tensor_tensor(
                out=q16[:], in0=q0[:], scalar=-MAGIC, in1=m[:],
                op0=Alu.add, op1=Alu.mult,
            )
            wqt_by_kt[kt] = q16
        wqt = [wqt_by_kt[kt] for kt in range(K_TILES)]

        # ---- bias slices (needed only by first eviction; issue late) ----
        bias_t = []
        for mi in range(M_TILES):
            bt = const.tile([P, 1], f32, tag=f"bias{mi}")
            nc.scalar.dma_start(bt[:, 0:1], bias[mi * P : (mi + 1) * P].unsqueeze(1))
            bias_t.append(bt)

        # ---- main matmul: yt[m, t] = sum_k wqt[k,m].T @ x16[k,t] ----
        for tb in range(N_TB):
            xh = {}
            for i, ki in enumerate(KT_ORDER):
                sl_p = slice(ki * P, (ki + 1) * P)
                sl_t = slice(tb * TB_TOK, (tb + 1) * TB_TOK)
                xht = x_p.tile([P, TB_TOK], f16, tag="xh", name=f"xh{tb}_{ki}")
                (nc.sync if i % 2 == 0 else nc.scalar).dma_start(xht[:], xth[sl_p, sl_t])
                xh[ki] = xht

            last_tb = tb == N_TB - 1

            def evict(mi, ps_tj):
                # 4 psum banks -> one [P, TB_TOK] fp16 tile, engines split by
                # tj parity (ACT/DVE hit different banks in parallel). On the
                # last token block, DMA per bank-pair so the drain overlaps
                # the final evictions instead of serializing after them.
                ysb = y_p.tile([P, TB_TOK], f16, tag="ysb", name=f"y{tb}_{mi}")
                for tj in range(TJ):
                    dst = ysb[:, tj * MM_N : (tj + 1) * MM_N]
                    if tj % 2 == 0:
                        nc.scalar.activation(
                            dst,
                            ps_tj[tj][:],
                            Act.Identity,
                            bias=bias_t[mi][:],
                            scale=s_t[:],
                        )
                    else:
                        nc.vector.tensor_scalar(
                            out=dst,
                            in0=ps_tj[tj][:],
                            scalar1=s_t[:],
                            scalar2=bias_t[mi][:],
                            op0=Alu.mult,
                            op1=Alu.add,
                        )
                    if last_tb and tj % 2 == 1:
                        tcol = tb * TB_TOK + (tj - 1) * MM_N
                        (nc.sync if (mi + tj) % 2 == 0 else nc.scalar).dma_start(
                            yt[mi * P : (mi + 1) * P, tcol : tcol + 2 * MM_N],
                            ysb[:, (tj - 1) * MM_N : (tj + 1) * MM_N],
                        )
                if not last_tb:
                    tcol = tb * TB_TOK
                    (nc.sync if mi % 2 == 0 else nc.scalar).dma_start(
                        yt[mi * P : (mi + 1) * P, tcol : tcol + TB_TOK], ysb[:]
                    )

            if tb == 0:
                # k-outer sweep: PE starts as soon as the first quantized
                # k-tile lands, consuming k-tiles at the prep pipeline's pace
                for mh in range(4):
                    ps = {
                        (ml, tj): psum_mm.tile(
                            [P, MM_N], f32, tag="ps", name=f"ps0_{mh}_{ml}_{tj}"
                        )
                        for ml in range(2)
                        for tj in range(TJ)
                    }
                    for i, ki in enumerate(KT_ORDER):
                        for ml in range(2):
                            mi = mh * 2 + ml
                            lhsT = wqt[ki][:, mi * P : (mi + 1) * P]
                            for tj in range(TJ):
                                nc.tensor.matmul(
                                    ps[ml, tj][:],
                                    lhsT,
                                    xh[ki][:, tj * MM_N : (tj + 1) * MM_N],
                                    start=(i == 0),
                                    stop=(i == K_TILES - 1),
                                )
                    for ml in range(2):
                        mi = mh * 2 + ml
                        evict(mi, [ps[ml, tj] for tj in range(TJ)])
            else:
                for mi in range(M_TILES):
                    ps = [
                        psum_mm.tile(
                            [P, MM_N], f32, tag="ps", name=f"ps{tb}_{mi}_{tj}"
                        )
                        for tj in range(TJ)
                    ]
                    for i, ki in enumerate(KT_ORDER):
                        lhsT = wqt[ki][:, mi * P : (mi + 1) * P]
                        for tj in range(TJ):
                            nc.tensor.matmul(
                                ps[tj][:],
                                lhsT,
                                xh[ki][:, tj * MM_N : (tj + 1) * MM_N],
                                start=(i == 0),
                                stop=(i == K_TILES - 1),
                            )
                    evict(mi, ps)

    nc.compile()
    return nc


def _get(qmax: float):
    key = qmax
    if key not in _CACHE:
        _CACHE[key] = _build(qmax)
    return _CACHE[key]


def host_prep(x, weight):
    """Host-side input re-encoding: transpose, phase-major permute the in_f
    axis, fp16 downcast of x. Pure layout/encoding; no module math."""
    xt = np.ascontiguousarray(x.T)[_PERM]  # [IN_F perm, TOKENS]
    xth = xt.astype(np.float16)
    wp = np.ascontiguousarray(weight.T[_PERM])  # [IN_F perm, OUT_F]
    return xth, wp


LAST_EXEC_NS = None


def kernel(x, weight, bias, precision, _trace_dir=None):
    global LAST_EXEC_NS
    from concourse.bass_utils import run_bass_kernel_spmd

    x = np.asarray(x, dtype=np.float32)
    weight = np.asarray(weight, dtype=np.float32)
    bias = np.asarray(bias, dtype=np.float32)
    prec = int(np.asarray(precision))
    qmax = float(2 ** (prec - 1) - 1)

    nc = _get(qmax)

    xth, wp = host_prep(x, weight)
    in_maps = [
        {
            "xth": np.ascontiguousarray(
                xth[:, c * TOK_PER_CORE : (c + 1) * TOK_PER_CORE]
            ),
            "wp": wp,
            "bias": bias,
        }
        for c in range(N_CORES)
    ]
    kw = {}
    if _trace_dir is not None:
        kw = {"trace": True, "tmpdir": _trace_dir}
    res = run_bass_kernel_spmd(nc, in_maps, list(range(N_CORES)), **kw)
    LAST_EXEC_NS = res.exec_time_ns
    yt = np.concatenate([res.results[c]["yt"] for c in range(N_CORES)], axis=1)
    return np.ascontiguousarray(yt.T.astype(np.float32))
